# revision 27
# baseline (speedup 1.0000x reference)
"""Balanced BCE loss on 8 Trainium2 NeuronCores.

loss = -sum_i [ beta_i * sum_j(t_ij * ln(p_ij))
                + (1-beta_i) * sum_j((1-t_ij) * ln(1-p_ij)) ]
beta_i = 1 - mean_j(t_ij)

Data-parallel: batch rows are sharded 8 per core; each core computes
per-row partial sums (A = sum t*ln p, B = sum ln(1-p), C = sum
t*ln(1-p)); the host combines them with beta (S = sum t per row is a
plain input reduction, done host-side in float64).

Per-core slab layout: the core's 8 rows (8MB contiguous HBM) are
viewed as [128, 16384] f32 where partition p owns the contiguous 64KB
line slab[p*16384:(p+1)*16384]; row r <-> partitions 16r..16r+15.

Three independent DMA queues interleave packet-wise at the SDMA
engines, covering each other's inter-transfer bubbles (measured
near-gapless at ~400GB/s effective):
  p-chunks: alternate across BOTH HWDGE rings (nc.sync / nc.scalar),
            f32 - p gets ~2/3 of the HBM bandwidth and finishes early,
            so ACT's serial Ln backlog clears before the stream ends
  t-chunks: SWDGE (nc.gpsimd) with f32->bf16 cast in the DMA datapath
            (removes the DVE cast op and halves t's SBUF footprint)

Streaming compute per column-chunk (DVE tensor_tensor runs 2x bf16
mode; the fused accum-reduce DVE variants only have 1x microcode and
tensor_tensor_reduce crashes this HW's exec unit, so the per-row
reductions ride the otherwise-idle PE instead):
  ACT: l1mp = Ln(1-p) bf16, accum_out -> accB[:,c]   (B for free)
       logp = Ln(p)  bf16
  DVE: m2 = t*l1mp, m1 = t*logp
  PE:  E^T @ {m2, m1} in <=512-col sub-blocks accumulated into
       psC/psA [8,512] PSUM across all chunks (E = block-indicator
       [128,8] host-provided constant; E[p,r]=1 iff p//16==r; a
       partition-offset memset is rejected by the BIR verifier, hence
       the tiny input tensor)

The chunk schedule tapers (1536, 512 last) so the serial chain after
the last DMA byte is short; the last chunk computes the logp/m1 path
first and the PSUM folds are split between ACT (psA via Copy+accum -
ScE reads PSUM cheaply) and DVE (psC, psB) so they overlap. Output is
padded to [8,128] f32: 512B per partition descriptor avoids the
sub-512B DMA read-modify-write penalty.

host: loss = -sum_rows[ beta*A + (1-beta)*(B-C) ], beta = 1-S/N
"""

from contextlib import ExitStack

import numpy as np

import concourse.bass as bass
import concourse.mybir as mybir
import concourse.tile as tile
from concourse import bacc
from concourse.bass_utils import run_bass_kernel_spmd

B, N = 64, 262144
NCORES = 8
ROWS = B // NCORES  # rows per core
P = 128  # SBUF partitions
F = ROWS * N // P  # 16384 cols per partition
GRP = P // ROWS  # 16 partitions per row

# column-chunk schedule: sums to F; mid-stream big (2MB p-DMAs), small last
CHUNKS = [2048, 4096, 4096, 2048, 2048, 1536, 512]
assert sum(CHUNKS) == F
NCH = len(CHUNKS)
CMAX = max(CHUNKS)
MM = 512  # matmul sub-block width (one PSUM bank; matmul cannot cross banks)
OUTW = 128  # padded stats width: 512B per partition descriptor

AF = mybir.ActivationFunctionType
ALU = mybir.AluOpType
f32 = mybir.dt.float32
bf16 = mybir.dt.bfloat16

# test.py can flip this to capture an NTFF profile of the run
TRACE = False
LAST = None  # BassKernelResults of the most recent kernel() call


def _emit(tc, out_ap, inp_ap, tgt_ap, emat_ap):
    nc = tc.nc

    with ExitStack() as ctx:
        singles = ctx.enter_context(tc.tile_pool(name="const", bufs=1))
        pch_pool = ctx.enter_context(tc.tile_pool(name="pch", bufs=5))
        tch_pool = ctx.enter_context(tc.tile_pool(name="tch", bufs=3))
        ln_pool = ctx.enter_context(tc.tile_pool(name="ln", bufs=3))
        mm_pool = ctx.enter_context(tc.tile_pool(name="mm", bufs=2))
        psum_pool = ctx.enter_context(tc.tile_pool(name="ps", bufs=1, space="PSUM"))

        accB = singles.tile([P, NCH], f32, tag="accB")
        junkps = singles.tile([ROWS, MM], bf16, tag="junkps")
        accBr = singles.tile([P, 1], f32, tag="accBr")
        ematf = singles.tile([P, ROWS], f32, tag="ematf")
        ematb = singles.tile([P, ROWS], bf16, tag="ematb")
        stats = singles.tile([ROWS, OUTW], f32, tag="stats")
        psA = psum_pool.tile([ROWS, MM], f32, tag="psA", name="psA")
        psC = psum_pool.tile([ROWS, MM], f32, tag="psC", name="psC")
        psB = psum_pool.tile([ROWS, 1], f32, tag="psB", name="psB")

        nc.gpsimd.memset(stats[:], 0.0)

        # slab views: [rows, n] -> [128, F], 64KB contiguous per partition
        inp3 = inp_ap.rearrange("r (a f) -> (r a) f", a=GRP)
        tgt3 = tgt_ap.rearrange("r (a f) -> (r a) f", a=GRP)

        offs = [0]
        for c in CHUNKS:
            offs.append(offs[-1] + c)

        # all DMA triggers upfront: p on the SP HWDGE ring, t via SWDGE
        # with inline f32->bf16 cast
        ptiles, ttiles = [], []
        for c in range(NCH):
            o, e = offs[c], offs[c + 1]
            pt = pch_pool.tile([P, CMAX], f32, tag="p", name=f"p{c}")
            # p alternates across BOTH HWDGE rings -> ~2/3 of the HBM
            # bandwidth while t (SWDGE) gets ~1/3: p lands early so ACT's
            # serial Ln backlog clears before the last t bytes arrive
            peng = nc.sync if c % 2 == 0 else nc.scalar
            peng.dma_start(pt[:, : e - o], inp3[:, o:e])
            ptiles.append(pt)
            tt = tch_pool.tile([P, CMAX], bf16, tag="t", name=f"t{c}")
            nc.gpsimd.dma_start(tt[:, : e - o], tgt3[:, o:e])
            ttiles.append(tt)
            if c == 0:
                nc.sync.dma_start(ematf[:], emat_ap)

        nc.vector.tensor_copy(ematb[:], ematf[:])

        def blocks(w):
            return [(b * MM, min(w, (b + 1) * MM)) for b in range((w + MM - 1) // MM)]

        nblk_total = sum(len(blocks(w)) for w in CHUNKS)
        nblk = 0
        for c in range(NCH):
            w = CHUNKS[c]
            last_chunk = c == NCH - 1
            p_t = ptiles[c][:, :w]
            t_t = ttiles[c][:, :w]

            if c >= NCH - 2:
                # dedicated tiles: the tail chunks' logs/products never
                # wait on pool recycling against downstream engines
                l1mp = singles.tile([P, w], bf16, tag=f"l1mp_t{c}")
                logp = singles.tile([P, w], bf16, tag=f"logp_t{c}")
                m2 = singles.tile([P, w], bf16, tag=f"m2_t{c}")
                m1 = singles.tile([P, w], bf16, tag=f"m1_t{c}")
            else:
                l1mp = ln_pool.tile([P, CMAX], bf16, tag="l1mp")
                logp = ln_pool.tile([P, CMAX], bf16, tag="logp")
                m2 = mm_pool.tile([P, CMAX], bf16, tag="m2")
                m1 = mm_pool.tile([P, CMAX], bf16, tag="m1")

            def act_l1mp():
                nc.scalar.activation(
                    l1mp[:, :w], p_t, AF.Ln, scale=-1.0, bias=1.0,
                    accum_out=accB[:, c : c + 1],
                )

            def act_logp():
                nc.scalar.activation(logp[:, :w], p_t, AF.Ln)

            # last chunk: logp/m1 path first so the psA chain (whose fold
            # is the longer, ACT one) closes as early as possible
            if last_chunk:
                act_logp(); act_l1mp()
                nc.vector.tensor_mul(m1[:, :w], t_t, logp[:, :w])
                nc.vector.tensor_mul(m2[:, :w], t_t, l1mp[:, :w])
                srcs, pss = (m1[:, :w], m2[:, :w]), (psA, psC)
            else:
                act_l1mp(); act_logp()
                nc.vector.tensor_mul(m2[:, :w], t_t, l1mp[:, :w])
                nc.vector.tensor_mul(m1[:, :w], t_t, logp[:, :w])
                srcs, pss = (m2[:, :w], m1[:, :w]), (psC, psA)

            for s, e in blocks(w):
                first, last = nblk == 0, nblk == nblk_total - 1
                for src, ps in zip(srcs, pss):
                    nc.tensor.matmul(ps[:, : e - s], ematb[:], src[:, s:e],
                                     start=first, stop=last)
                nblk += 1

        # epilogue: folds split across ACT and DVE so they overlap.
        # ACT: psA via Copy+accum (PSUM reads are cheap on ScE)
        # DVE: accB fold, psC fold, psB copy;  PE: B row-sums matmul
        nc.scalar.activation(junkps[:], psA[:], AF.Copy, accum_out=stats[:, 2:3])
        nc.vector.tensor_reduce(accBr[:], accB[:], axis=mybir.AxisListType.X, op=ALU.add)
        nc.tensor.matmul(psB[:], ematf[:], accBr[:])
        nc.vector.tensor_reduce(stats[:, 3:4], psC[:], axis=mybir.AxisListType.X, op=ALU.add)
        nc.vector.tensor_copy(stats[:, 1:2], psB[:])
        nc.sync.dma_start(out_ap, stats[:])


_PROG_CACHE = {}


def _build_program():
    key = "v11"
    if key not in _PROG_CACHE:
        nc = bacc.Bacc("TRN2", target_bir_lowering=False, debug=False)
        inp = nc.dram_tensor("input", [ROWS, N], f32, kind="ExternalInput").ap()
        tgt = nc.dram_tensor("target", [ROWS, N], f32, kind="ExternalInput").ap()
        emat = nc.dram_tensor("emat", [P, ROWS], f32, kind="ExternalInput").ap()
        out = nc.dram_tensor("partials", [ROWS, OUTW], f32, kind="ExternalOutput").ap()
        with tile.TileContext(nc) as tc:
            _emit(tc, out, inp, tgt, emat)
        nc.finalize()
        _PROG_CACHE[key] = nc
    return _PROG_CACHE[key]


def _emat_np():
    e = np.zeros((P, ROWS), dtype=np.float32)
    for r in range(ROWS):
        e[r * GRP : (r + 1) * GRP, r] = 1.0
    return e


def kernel(input, target):
    global LAST
    input = np.ascontiguousarray(np.asarray(input))
    target = np.ascontiguousarray(np.asarray(target))
    assert input.shape == (B, N) and target.shape == (B, N)

    nc = _build_program()
    emat = _emat_np()
    in_maps = [
        {
            "input": input[c * ROWS : (c + 1) * ROWS],
            "target": target[c * ROWS : (c + 1) * ROWS],
            "emat": emat,
        }
        for c in range(NCORES)
    ]
    res = run_bass_kernel_spmd(nc, in_maps, core_ids=list(range(NCORES)), trace=TRACE)
    LAST = res

    # S (= sum of t per row) is a plain reduction of an input tensor;
    # the host computes it in float64 alongside the beta combine
    tsum = target.astype(np.float64).sum(axis=1)
    total = np.float64(0.0)
    for c in range(NCORES):
        part = res.results[c]["partials"].astype(np.float64)  # [ROWS, OUTW]
        Bv, A, C = part[:, 1], part[:, 2], part[:, 3]
        S = tsum[c * ROWS : (c + 1) * ROWS]
        beta = 1.0 - S / N
        total += np.sum(beta * A + (1.0 - beta) * (Bv - C))
    return np.float32(-total)


# revision 28
# speedup vs baseline: 1.0375x; 1.0375x over previous
"""Balanced BCE loss on 8 Trainium2 NeuronCores.

loss = -sum_i [ beta_i * sum_j(t_ij * ln(p_ij))
                + (1-beta_i) * sum_j((1-t_ij) * ln(1-p_ij)) ]
beta_i = 1 - mean_j(t_ij)

Data-parallel: batch rows are sharded 8 per core; each core computes
per-row partial sums (A = sum t*ln p, B = sum ln(1-p), C = sum
t*ln(1-p)); the host combines them with beta (S = sum t per row is a
plain input reduction, done host-side in float64).

Per-core slab layout: the core's 8 rows (8MB contiguous HBM) are
viewed as [128, 16384] f32 where partition p owns the contiguous 64KB
line slab[p*16384:(p+1)*16384]; row r <-> partitions 16r..16r+15.

Three independent DMA queues interleave packet-wise at the SDMA
engines, covering each other's inter-transfer bubbles (measured
near-gapless at ~400GB/s effective):
  p-chunks: alternate across BOTH HWDGE rings (nc.sync / nc.scalar),
            f32 - p gets ~2/3 of the HBM bandwidth and finishes early,
            so ACT's serial Ln backlog clears before the stream ends
  t-chunks: SWDGE (nc.gpsimd) with f32->bf16 cast in the DMA datapath
            (removes the DVE cast op and halves t's SBUF footprint)

Streaming compute per column-chunk (DVE tensor_tensor runs 2x bf16
mode; the fused accum-reduce DVE variants only have 1x microcode and
tensor_tensor_reduce crashes this HW's exec unit, so the per-row
reductions ride the otherwise-idle PE instead):
  ACT: l1mp = Ln(1-p) bf16, accum_out -> accB[:,c]   (B for free)
       logp = Ln(p)  bf16
  DVE: m2 = t*l1mp, m1 = t*logp
  PE:  E^T @ {m2, m1} in <=512-col sub-blocks accumulated into
       psC/psA [8,512] PSUM across all chunks (E = block-indicator
       [128,8] host-provided constant; E[p,r]=1 iff p//16==r; a
       partition-offset memset is rejected by the BIR verifier, hence
       the tiny input tensor)

The chunk schedule tapers (1536, 512 last) so the serial chain after
the last DMA byte is short; the last chunk computes the logp/m1 path
first and the PSUM folds are split between ACT (psA via Copy+accum -
ScE reads PSUM cheaply) and DVE (psC, psB) so they overlap. Output is
padded to [8,128] f32: 512B per partition descriptor avoids the
sub-512B DMA read-modify-write penalty.

host: loss = -sum_rows[ beta*A + (1-beta)*(B-C) ], beta = 1-S/N
"""

from contextlib import ExitStack

import numpy as np

import concourse.bass as bass
import concourse.mybir as mybir
import concourse.tile as tile
from concourse import bacc
from concourse.bass_utils import run_bass_kernel_spmd

B, N = 64, 262144
NCORES = 8
ROWS = B // NCORES  # rows per core
P = 128  # SBUF partitions
F = ROWS * N // P  # 16384 cols per partition
GRP = P // ROWS  # 16 partitions per row

# column-chunk schedule: sums to F; mid-stream big (2MB p-DMAs), small last
CHUNKS = [2048, 4096, 4096, 2048, 2048, 1536, 512]
assert sum(CHUNKS) == F
NCH = len(CHUNKS)
CMAX = max(CHUNKS)
MM = 512  # matmul sub-block width (one PSUM bank; matmul cannot cross banks)
OUTW = 128  # padded stats width: 512B per partition descriptor

AF = mybir.ActivationFunctionType
ALU = mybir.AluOpType
f32 = mybir.dt.float32
bf16 = mybir.dt.bfloat16

# test.py can flip this to capture an NTFF profile of the run
TRACE = False
LAST = None  # BassKernelResults of the most recent kernel() call


def _emit(tc, out_ap, inp_ap, tgt_ap, emat_ap):
    nc = tc.nc

    with ExitStack() as ctx:
        singles = ctx.enter_context(tc.tile_pool(name="const", bufs=1))
        pch_pool = ctx.enter_context(tc.tile_pool(name="pch", bufs=5))
        tch_pool = ctx.enter_context(tc.tile_pool(name="tch", bufs=3))
        ln_pool = ctx.enter_context(tc.tile_pool(name="ln", bufs=3))
        mm_pool = ctx.enter_context(tc.tile_pool(name="mm", bufs=2))
        psum_pool = ctx.enter_context(tc.tile_pool(name="ps", bufs=1, space="PSUM"))

        accB = singles.tile([P, NCH], f32, tag="accB")
        junkps = singles.tile([ROWS, MM], bf16, tag="junkps")
        accBr = singles.tile([P, 1], f32, tag="accBr")
        ematf = singles.tile([P, ROWS], f32, tag="ematf")
        ematb = singles.tile([P, ROWS], bf16, tag="ematb")
        stats = singles.tile([ROWS, OUTW], f32, tag="stats")
        psA = psum_pool.tile([ROWS, MM], f32, tag="psA", name="psA")
        psC = psum_pool.tile([ROWS, MM], f32, tag="psC", name="psC")
        psB = psum_pool.tile([ROWS, 1], f32, tag="psB", name="psB")

        nc.gpsimd.memset(stats[:], 0.0)

        # slab views: [rows, n] -> [128, F], 64KB contiguous per partition
        inp3 = inp_ap.rearrange("r (a f) -> (r a) f", a=GRP)
        tgt3 = tgt_ap.rearrange("r (a f) -> (r a) f", a=GRP)

        offs = [0]
        for c in CHUNKS:
            offs.append(offs[-1] + c)

        # all DMA triggers upfront: p on the SP HWDGE ring, t via SWDGE
        # with inline f32->bf16 cast
        ptiles, ttiles = [], []
        for c in range(NCH):
            o, e = offs[c], offs[c + 1]
            pt = pch_pool.tile([P, CMAX], f32, tag="p", name=f"p{c}")
            # p alternates across BOTH HWDGE rings -> ~2/3 of the HBM
            # bandwidth while t (SWDGE) gets ~1/3: p lands early so ACT's
            # serial Ln backlog clears before the last t bytes arrive
            peng = nc.sync if c % 2 == 0 else nc.scalar
            peng.dma_start(pt[:, : e - o], inp3[:, o:e])
            ptiles.append(pt)
            tt = tch_pool.tile([P, CMAX], bf16, tag="t", name=f"t{c}")
            if c == NCH - 1:
                # split the final t transfer: the first half's multiply
                # overlaps the second half's transfer + completion receipt
                h = (e - o) // 2
                nc.gpsimd.dma_start(tt[:, :h], tgt3[:, o : o + h])
                nc.gpsimd.dma_start(tt[:, h : e - o], tgt3[:, o + h : e])
            else:
                nc.gpsimd.dma_start(tt[:, : e - o], tgt3[:, o:e])
            ttiles.append(tt)
            if c == 0:
                nc.sync.dma_start(ematf[:], emat_ap)

        nc.vector.tensor_copy(ematb[:], ematf[:])

        def blocks(w):
            return [(b * MM, min(w, (b + 1) * MM)) for b in range((w + MM - 1) // MM)]

        nblk_total = sum(len(blocks(w)) for w in CHUNKS)
        nblk = 0
        for c in range(NCH):
            w = CHUNKS[c]
            last_chunk = c == NCH - 1
            p_t = ptiles[c][:, :w]
            t_t = ttiles[c][:, :w]

            if c >= NCH - 2:
                # dedicated tiles: the tail chunks' logs/products never
                # wait on pool recycling against downstream engines
                l1mp = singles.tile([P, w], bf16, tag=f"l1mp_t{c}")
                logp = singles.tile([P, w], bf16, tag=f"logp_t{c}")
                m2 = singles.tile([P, w], bf16, tag=f"m2_t{c}")
                m1 = singles.tile([P, w], bf16, tag=f"m1_t{c}")
            else:
                l1mp = ln_pool.tile([P, CMAX], bf16, tag="l1mp")
                logp = ln_pool.tile([P, CMAX], bf16, tag="logp")
                m2 = mm_pool.tile([P, CMAX], bf16, tag="m2")
                m1 = mm_pool.tile([P, CMAX], bf16, tag="m1")

            def act_l1mp():
                nc.scalar.activation(
                    l1mp[:, :w], p_t, AF.Ln, scale=-1.0, bias=1.0,
                    accum_out=accB[:, c : c + 1],
                )

            def act_logp():
                nc.scalar.activation(logp[:, :w], p_t, AF.Ln)

            # last chunk: logp/m1 path first so the psA chain (whose fold
            # is the longer, ACT one) closes as early as possible
            if last_chunk:
                act_logp(); act_l1mp()
                h = w // 2
                nc.vector.tensor_mul(m1[:, :h], t_t[:, :h], logp[:, :h])
                nc.vector.tensor_mul(m1[:, h:w], t_t[:, h:w], logp[:, h:w])
                nc.vector.tensor_mul(m2[:, :h], t_t[:, :h], l1mp[:, :h])
                nc.vector.tensor_mul(m2[:, h:w], t_t[:, h:w], l1mp[:, h:w])
                srcs, pss = (m1[:, :w], m2[:, :w]), (psA, psC)
            else:
                act_l1mp(); act_logp()
                nc.vector.tensor_mul(m2[:, :w], t_t, l1mp[:, :w])
                nc.vector.tensor_mul(m1[:, :w], t_t, logp[:, :w])
                srcs, pss = (m2[:, :w], m1[:, :w]), (psC, psA)

            for s, e in blocks(w):
                first, last = nblk == 0, nblk == nblk_total - 1
                for src, ps in zip(srcs, pss):
                    nc.tensor.matmul(ps[:, : e - s], ematb[:], src[:, s:e],
                                     start=first, stop=last)
                nblk += 1

        # epilogue: folds split across ACT and DVE so they overlap.
        # ACT: psA via Copy+accum (PSUM reads are cheap on ScE)
        # DVE: accB fold, psC fold, psB copy;  PE: B row-sums matmul
        nc.scalar.activation(junkps[:], psA[:], AF.Copy, accum_out=stats[:, 2:3])
        nc.vector.tensor_reduce(accBr[:], accB[:], axis=mybir.AxisListType.X, op=ALU.add)
        nc.tensor.matmul(psB[:], ematf[:], accBr[:])
        nc.vector.tensor_reduce(stats[:, 3:4], psC[:], axis=mybir.AxisListType.X, op=ALU.add)
        nc.vector.tensor_copy(stats[:, 1:2], psB[:])
        nc.sync.dma_start(out_ap, stats[:])


_PROG_CACHE = {}


def _build_program():
    key = "v12"
    if key not in _PROG_CACHE:
        nc = bacc.Bacc("TRN2", target_bir_lowering=False, debug=False)
        inp = nc.dram_tensor("input", [ROWS, N], f32, kind="ExternalInput").ap()
        tgt = nc.dram_tensor("target", [ROWS, N], f32, kind="ExternalInput").ap()
        emat = nc.dram_tensor("emat", [P, ROWS], f32, kind="ExternalInput").ap()
        out = nc.dram_tensor("partials", [ROWS, OUTW], f32, kind="ExternalOutput").ap()
        with tile.TileContext(nc) as tc:
            _emit(tc, out, inp, tgt, emat)
        nc.finalize()
        _PROG_CACHE[key] = nc
    return _PROG_CACHE[key]


def _emat_np():
    e = np.zeros((P, ROWS), dtype=np.float32)
    for r in range(ROWS):
        e[r * GRP : (r + 1) * GRP, r] = 1.0
    return e


def kernel(input, target):
    global LAST
    input = np.ascontiguousarray(np.asarray(input))
    target = np.ascontiguousarray(np.asarray(target))
    assert input.shape == (B, N) and target.shape == (B, N)

    nc = _build_program()
    emat = _emat_np()
    in_maps = [
        {
            "input": input[c * ROWS : (c + 1) * ROWS],
            "target": target[c * ROWS : (c + 1) * ROWS],
            "emat": emat,
        }
        for c in range(NCORES)
    ]
    res = run_bass_kernel_spmd(nc, in_maps, core_ids=list(range(NCORES)), trace=TRACE)
    LAST = res

    # S (= sum of t per row) is a plain reduction of an input tensor;
    # the host computes it in float64 alongside the beta combine
    tsum = target.astype(np.float64).sum(axis=1)
    total = np.float64(0.0)
    for c in range(NCORES):
        part = res.results[c]["partials"].astype(np.float64)  # [ROWS, OUTW]
        Bv, A, C = part[:, 1], part[:, 2], part[:, 3]
        S = tsum[c * ROWS : (c + 1) * ROWS]
        beta = 1.0 - S / N
        total += np.sum(beta * A + (1.0 - beta) * (Bv - C))
    return np.float32(-total)
